# revision 8
# baseline (speedup 1.0000x reference)
"""GQA causal attention block (B=2, L=2048, d_model=2048, 32 Q heads / 8 KV heads)
on 8 TRN2 NeuronCores.

Round-3 kernel: software-pipelined attention inner loop (score matmuls of step
k+1 issue before AV of step k so the PE never waits on the scalar-engine exp),
causal column restriction on the AV matmuls (no dead-zone memsets), exp merged
into 1-2 wide ACTIVATE instrs per step, phase-4 normalization prep overlapped
with attention / the second AllToAll, and finer first-chunk x DMA.

Baseline (round 2) measured 468 us on HW; see kernel_r2_backup.py.
"""

import os
import sys
import math

os.environ.setdefault("MYCRO_LOCAL_CACHE", "1")
for _p in ("/opt/trn_rl_repo",):
    if os.path.isdir(_p) and _p not in sys.path:
        sys.path.insert(0, _p)

import numpy as np
import ml_dtypes

import concourse.bass as bass
import concourse.bacc as bacc
import concourse.mybir as mybir
import concourse.tile as tile
from concourse.bass_utils import run_bass_kernel_spmd
from concourse.masks import make_identity

F32 = mybir.dt.float32
BF16 = mybir.dt.bfloat16
Exp = mybir.ActivationFunctionType.Exp

D = 2048
L = 2048
DH = 64
B = 2
NCORES = 8
NH_L = 4
QF = NH_L * DH
LC1 = 512
NLC1 = L // LC1
LT = 512
NT = L // LT
NB = L // 128
SH = 2 * DH  # 128 rows per A2A shard (normalized, no denom rows)
SCALE = 1.0 / math.sqrt(DH)

_CACHE = {}


def _build_nc():
    nc = bacc.Bacc(
        "TRN2",
        target_bir_lowering=False,
        debug=False,
        enable_asserts=False,
        num_devices=NCORES,
    )
    xh0 = nc.dram_tensor("xh0", [NLC1 * 128, 16 * LC1], BF16, kind="ExternalInput")
    xh1 = nc.dram_tensor("xh1", [NLC1 * 128, 16 * LC1], BF16, kind="ExternalInput")
    wqh = nc.dram_tensor("wqh", [128, 16 * QF], BF16, kind="ExternalInput")
    wkvh = nc.dram_tensor("wkvh", [128, 16 * 256], BF16, kind="ExternalInput")
    woh = nc.dram_tensor("woh", [D, D], BF16, kind="ExternalInput")
    y = nc.dram_tensor("y", [LT, D], F32, kind="ExternalOutput")

    with tile.TileContext(nc) as tc:
        with tc.tile_pool(name="dram", bufs=1, space="DRAM") as dram:
            bins = [
                dram.tile([NCORES * SH, LT], BF16, name=f"bin{hp}") for hp in range(2)
            ]
            bouts = [
                dram.tile([NCORES * SH, LT], BF16, name=f"bout{hp}") for hp in range(2)
            ]
            rnorms = [
                dram.tile([8, 1024], BF16, name=f"rnorm{hp}") for hp in range(2)
            ]

            with tc.tile_pool(name="const", bufs=1) as const:
                ident = const.tile([128, 128], BF16, name="ident")
                make_identity(nc, ident)

                with tc.tile_pool(name="pers", bufs=1) as pers:
                    qp = [
                        pers.tile([128, NT * 1024], BF16, name=f"qp{p}")
                        for p in range(2)
                    ]
                    kbT = pers.tile([128, L], BF16, name="kbT")
                    vaug = pers.tile([128, NB * 130], BF16, name="vaug")
                    va = vaug.rearrange("p (b c) -> p b c", c=130)
                    nc.gpsimd.memset(va[:, :, 64:65], 1.0)
                    nc.gpsimd.memset(va[:, :, 129:130], 1.0)

                    _phase1_qkv(nc, tc, xh0, xh1, wqh, wkvh, qp, kbT, va, ident)
                    with tc.tile_pool(name="wo", bufs=1) as wop:
                        wo_sb = wop.tile([128, 16 * D], BF16, name="wo_sb")
                        nc.gpsimd.dma_start(
                            wo_sb.rearrange("p (k d) -> p k d", d=D),
                            woh.rearrange("(k p) d -> p k d", p=128),
                        )
                        with tc.tile_pool(name="an", bufs=1) as anp:
                            ans = {}
                            with (
                                tc.tile_pool(name="p2s", bufs=1, space="PSUM") as scp,
                                tc.tile_pool(name="p2o", bufs=1, space="PSUM") as ovp,
                                tc.tile_pool(name="pbuf", bufs=1) as pbp,
                                tc.tile_pool(name="stg", bufs=3) as stp,
                            ):
                                pools = (scp, ovp, pbp, stp)
                                _attn_pair(nc, tc, pools, qp, kbT, va, bins[0], rnorms[0], 0)
                                nc.gpsimd.collective_compute(
                                    "AllToAll",
                                    mybir.AluOpType.bypass,
                                    ins=[bins[0].opt()],
                                    outs=[bouts[0].opt()],
                                    replica_groups=[list(range(NCORES))],
                                )
                                _attn_pair(
                                    nc, tc, pools, qp, kbT, va, bins[1], rnorms[1], 1,
                                    after_tau={
                                        1: lambda: _prep_half(
                                            nc, anp, bouts, 0, ans
                                        )
                                    },
                                )
                                nc.gpsimd.collective_compute(
                                    "AllToAll",
                                    mybir.AluOpType.bypass,
                                    ins=[bins[1].opt()],
                                    outs=[bouts[1].opt()],
                                    replica_groups=[list(range(NCORES))],
                                )
                            # attention PSUM pools released here
                            _prep_half(nc, anp, bouts, 1, ans)
                            _phase4_oproj(nc, tc, wo_sb, ans, y)
    nc.finalize()
    return nc


def _phase1_qkv(nc, tc, xh0, xh1, wqh, wkvh, qp, kbT, va, ident):
    with (
        tc.tile_pool(name="w1", bufs=1) as wpool,
        tc.tile_pool(name="xc", bufs=2) as xpool,
        tc.tile_pool(name="vt", bufs=2) as vtpool,
        tc.tile_pool(name="p1q", bufs=1, space="PSUM") as p1q,
        tc.tile_pool(name="p1kv", bufs=1, space="PSUM") as p1kv,
    ):
        wq_sb = wpool.tile([128, 16 * QF], BF16, name="wq_sb")
        wkv_sb = wpool.tile([128, 16 * 256], BF16, name="wkv_sb")
        nc.sync.dma_start(wq_sb[:, :], wqh[:, :])
        nc.sync.dma_start(wkv_sb[:, :], wkvh[:, :])

        for lc in range(NLC1):
            x0 = xpool.tile([128, 16 * LC1], BF16, name="x0", tag="x0")
            x1 = xpool.tile([128, 16 * LC1], BF16, name="x1", tag="x1")
            # finer-grained parts for the first chunk so the first matmul
            # can start as soon as one fb-column block has landed
            nparts = 16 if lc == 0 else 4
            pstep = (16 * LC1) // nparts
            for part in range(nparts):
                cs = slice(part * pstep, (part + 1) * pstep)
                nc.gpsimd.dma_start(x0[:, cs], xh0[lc * 128 : (lc + 1) * 128, cs])
                nc.gpsimd.dma_start(x1[:, cs], xh1[lc * 128 : (lc + 1) * 128, cs])
            cols = slice(lc * LC1, (lc + 1) * LC1)

            aq = [
                [
                    p1q.tile([128, LC1], F32, name=f"aq{p}{b}", tag=f"aq{p}{b}")
                    for b in range(2)
                ]
                for p in range(2)
            ]
            akv = [
                p1kv.tile([128, LC1], F32, name=f"akv{b}", tag=f"akv{b}")
                for b in range(2)
            ]
            for fb in range(16):
                st = dict(start=(fb == 0), stop=(fb == 15))
                x0f = x0[:, fb * LC1 : (fb + 1) * LC1]
                x1f = x1[:, fb * LC1 : (fb + 1) * LC1]
                for p in range(2):
                    w = wq_sb[:, fb * QF + p * 128 : fb * QF + (p + 1) * 128]
                    nc.tensor.matmul(aq[p][0][:, :], w, x0f, **st)
                    nc.tensor.matmul(aq[p][1][:, :], w, x1f, **st)
                wkv0 = wkv_sb[:, fb * 256 : fb * 256 + 128]
                wkv1 = wkv_sb[:, fb * 256 + 128 : (fb + 1) * 256]
                nc.tensor.matmul(akv[0][:, :], wkv0, x0f, **st)
                nc.tensor.matmul(akv[1][:, :], wkv1, x1f, **st)

            for p in range(2):
                qc = lc * 1024
                nc.scalar.copy(qp[p][0:64, qc : qc + 512], aq[p][0][0:64, :])
                nc.scalar.copy(qp[p][64:128, qc : qc + 512], aq[p][1][0:64, :])
                nc.scalar.copy(
                    qp[p][64:128, qc + 512 : qc + 1024], aq[p][1][64:128, :]
                )
                tq = vtpool.tile([128, LC1], BF16, name="tq", tag=f"tq{p}")
                nc.scalar.copy(tq[64:128, :], aq[p][0][64:128, :])
                nc.sync.dma_start(qp[p][0:64, qc + 512 : qc + 1024], tq[64:128, :])
            nc.scalar.copy(kbT[0:64, cols], akv[0][0:64, :])
            nc.scalar.copy(kbT[64:128, cols], akv[1][64:128, :])
            vsb = vtpool.tile([128, LC1], BF16, name="vsb", tag="vsb")
            nc.scalar.copy(vsb[0:64, :], akv[1][0:64, :])
            nc.scalar.copy(vsb[64:128, :], akv[0][64:128, :])
            for s in range(LC1 // 128):
                beta = (lc * LC1) // 128 + s
                tp = p1kv.tile([128, 128], BF16, name="tp", tag="tp", bufs=2)
                nc.tensor.matmul(
                    tp[:, :],
                    vsb[:, s * 128 : (s + 1) * 128],
                    ident[:, :],
                    is_transpose=True,
                )
                nc.scalar.copy(va[:, beta, 0:64], tp[:, 64:128])
                nc.scalar.copy(va[:, beta, 65:129], tp[:, 0:64])


def _attn_pair(nc, tc, pools, qp, kbT, va, bin_, rnorm, hp, after_tau=None):
    """Attention for head pair hp (2 heads x 2 batches), software-pipelined:
    the score matmuls for step k+1 are issued before the AV matmuls of step
    k, so the tensor engine streams while the scalar engine computes exp."""
    scp, ovp, pbp, stp = pools

    steps = []
    for tau in range(NT):
        nb = 4 * tau + 4
        for blk in range(nb):
            for h in (0, 1):
                steps.append((tau, blk, h))

    outs = {}

    def get_out(tau):
        if tau not in outs:
            outs[tau] = (
                ovp.tile([65, 1024], F32, name="oab", tag="oab"),
                ovp.tile([65, 1024], F32, name="obb", tag="obb"),
            )
        return outs[tau]

    def emit_S(step):
        tau, blk, h = step
        off = max(blk - 4 * tau, 0) * 128
        kc = slice(blk * 128, (blk + 1) * 128)
        tq = tau * 1024
        sb = scp.tile([128, 1024], F32, name="sb", tag="sb", bufs=2)
        nc.tensor.matmul(
            sb[:, off:512],
            kbT[64 * h : 64 * (h + 1), kc],
            qp[hp][64 * h : 64 * (h + 1), tq + off : tq + 512],
        )
        nc.tensor.matmul(
            sb[:, 512 + off : 1024],
            kbT[64 * h : 64 * (h + 1), kc],
            qp[hp][64 * h : 64 * (h + 1), tq + 512 + off : tq + 1024],
            skip_group_check=True,
        )
        return sb

    def emit_exp(step, sb):
        tau, blk, h = step
        dj = blk - 4 * tau
        off = max(dj, 0) * 128
        Pb = pbp.tile([128, 1024], BF16, name="Pb", tag="Pb", bufs=3)
        if off == 0:
            nc.scalar.activation(Pb[:, :], sb[:, :], Exp)
        else:
            nc.scalar.activation(Pb[:, off:512], sb[:, off:512], Exp)
            nc.scalar.activation(
                Pb[:, 512 + off : 1024], sb[:, 512 + off : 1024], Exp
            )
        if dj >= 0:
            for base in (0, 512):
                dg = Pb[:, base + off : base + off + 128]
                nc.gpsimd.affine_select(
                    out=dg,
                    in_=dg,
                    compare_op=mybir.AluOpType.is_ge,
                    fill=0.0,
                    base=0,
                    pattern=[[1, 128]],
                    channel_multiplier=-1,
                )
        return Pb

    def emit_AV(step, Pb):
        tau, blk, h = step
        nb = 4 * tau + 4
        off = max(blk - 4 * tau, 0) * 128
        out = get_out(tau)[h]
        st = dict(start=(blk == 0), stop=(blk == nb - 1))
        nc.tensor.matmul(
            out[:, off:512],
            va[:, blk, 65 * h : 65 * h + 65],
            Pb[:, off:512],
            skip_group_check=True,
            **st,
        )
        nc.tensor.matmul(
            out[:, 512 + off : 1024],
            va[:, blk, 65 * h : 65 * h + 65],
            Pb[:, 512 + off : 1024],
            skip_group_check=True,
            **st,
        )

    def emit_evac(tau):
        oab, obb = outs.pop(tau)
        for bb, psrc in ((0, oab), (1, obb)):
            # snapshot PSUM accumulators to SBUF so the PSUM tile rotation is
            # only gated by these two fast copies; normalize off to the side
            snapd = stp.tile([1, 1024], F32, name="snapd", tag=f"snapd{bb}", bufs=2)
            nc.vector.tensor_copy(snapd[:, :], psrc[64:65, :])
            snapb = stp.tile([64, 1024], BF16, name="snapb", tag=f"snapb{bb}", bufs=2)
            nc.vector.tensor_copy(snapb[:, :], psrc[0:64, :])
            rrow = stp.tile([1, 1024], F32, name="rrow", tag=f"rrow{bb}", bufs=2)
            nc.vector.reciprocal(rrow[:, :], snapd[:, :])
            rrowb = stp.tile([1, 1024], BF16, name="rrowb", tag=f"rrowb{bb}", bufs=2)
            nc.vector.tensor_copy(rrowb[:, :], rrow[:, :])
            rr = 2 * tau + bb
            nc.sync.dma_start(rnorm[rr : rr + 1, :], rrowb[0:1, :])
            rbc = stp.tile([64, 1024], BF16, name="rbc", tag=f"rbc{bb}", bufs=2)
            nc.sync.dma_start(
                rbc[:, :], rnorm[rr : rr + 1, :].partition_broadcast(64)
            )
            stgn = stp.tile([64, 1024], BF16, name="stgn", tag=f"stgn{bb}", bufs=2)
            nc.vector.tensor_mul(stgn[:, :], snapb[:, :], rbc[:, :])
            base = SH * (4 * bb + tau)
            for hh in (0, 1):
                nc.sync.dma_start(
                    bin_[base + 64 * hh : base + 64 * (hh + 1), :],
                    stgn[:, 512 * hh : 512 * (hh + 1)],
                )

    sb_cur = emit_S(steps[0])
    for i, step in enumerate(steps):
        Pb = emit_exp(step, sb_cur)
        if i + 1 < len(steps):
            sb_cur = emit_S(steps[i + 1])
        emit_AV(step, Pb)
        tau, blk, h = step
        nb = 4 * tau + 4
        if blk == nb - 1 and h == 1:
            emit_evac(tau)
            if after_tau and tau in after_tau:
                after_tau[tau]()


def _prep_half(nc, anp, bouts, half, ans):
    """Load one A2A result into SBUF: the numerators arrive already
    normalized, so this is just 8 plain DMA loads on the sync queue."""
    bo = bouts[half]
    for c in range(NCORES):
        k = 2 * c + half
        au = anp.tile([128, LT], BF16, name=f"au{k}", tag=f"au{k}")
        nc.sync.dma_start(au[:, :], bo[SH * c : SH * (c + 1), :])
        ans[k] = au


def _phase4_oproj(nc, tc, wo_sb, ans, y):
    with (
        tc.tile_pool(name="ysum", bufs=1) as ysp,
        tc.tile_pool(name="ysb", bufs=2) as yp,
        tc.tile_pool(name="p4y", bufs=2, space="PSUM") as eyp,
    ):
        ysum = [
            ysp.tile([128, D], F32, name=f"ysum{m}", tag=f"ysum{m}")
            for m in range(4)
        ]
        for half in range(2):
            for m in range(4):
                yps = eyp.tile([128, D], F32, name="yps", tag="yps")
                for ki in range(NCORES):
                    k = 2 * ki + half
                    st = dict(start=(ki == 0), stop=(ki == NCORES - 1))
                    for q in range(4):
                        nc.tensor.matmul(
                            yps[:, q * 512 : (q + 1) * 512],
                            ans[k][:, m * 128 : (m + 1) * 128],
                            wo_sb[:, k * D + q * 512 : k * D + (q + 1) * 512],
                            skip_group_check=(q > 0),
                            **st,
                        )
                if half == 0:
                    nc.scalar.copy(ysum[m][:, :], yps[:, :])
                else:
                    ysb = yp.tile([128, D], F32, name="ysb", tag="ysb")
                    nc.vector.tensor_add(ysb[:, :], yps[:, :], ysum[m][:, :])
                    nc.sync.dma_start(y[m * 128 : (m + 1) * 128, :], ysb[:, :])


def _get_nc():
    if "nc" not in _CACHE:
        _CACHE["nc"] = _build_nc()
    return _CACHE["nc"]


LAST_EXEC_NS = None


def _prep_x(xb):
    xT = xb.T.astype(ml_dtypes.bfloat16)
    v = xT.reshape(16, 128, NLC1, LC1)
    v = v.transpose(2, 1, 0, 3)
    return np.ascontiguousarray(v.reshape(NLC1 * 128, 16 * LC1))


def kernel(x, Wq, Wk, Wv, Wo):
    global LAST_EXEC_NS
    x = np.asarray(x, dtype=np.float32)
    Wq = np.asarray(Wq, dtype=np.float32)
    Wk = np.asarray(Wk, dtype=np.float32)
    Wv = np.asarray(Wv, dtype=np.float32)
    Wo = np.asarray(Wo, dtype=np.float32)

    xh0 = _prep_x(x[0])
    xh1 = _prep_x(x[1])
    woh = np.ascontiguousarray(Wo.T.astype(ml_dtypes.bfloat16))

    in_maps = []
    for c in range(NCORES):
        wq_c = (SCALE * Wq[QF * c : QF * (c + 1), :]).astype(ml_dtypes.bfloat16)
        wqh = np.ascontiguousarray(
            wq_c.reshape(QF, 16, 128).transpose(2, 1, 0).reshape(128, 16 * QF)
        )
        wk_c = Wk[DH * c : DH * (c + 1), :].astype(ml_dtypes.bfloat16)
        wkh = wk_c.reshape(DH, 16, 128).transpose(2, 1, 0)
        wv_c = Wv[DH * c : DH * (c + 1), :].astype(ml_dtypes.bfloat16)
        wvh = wv_c.reshape(DH, 16, 128).transpose(2, 1, 0)
        wkvh = np.empty((128, 16, 256), dtype=ml_dtypes.bfloat16)
        wkvh[:, :, 0:64] = wkh
        wkvh[:, :, 64:128] = wvh
        wkvh[:, :, 128:192] = wvh
        wkvh[:, :, 192:256] = wkh
        wkvh = np.ascontiguousarray(wkvh.reshape(128, 16 * 256))
        in_maps.append(
            {"xh0": xh0, "xh1": xh1, "wqh": wqh, "wkvh": wkvh, "woh": woh}
        )

    nc = _get_nc()
    res = run_bass_kernel_spmd(nc, in_maps, core_ids=list(range(NCORES)))
    LAST_EXEC_NS = getattr(res, "exec_time_ns", None)

    out = np.empty((B, L, D), dtype=np.float32)
    for c in range(NCORES):
        b, g = divmod(c, 4)
        out[b, 512 * g : 512 * (g + 1), :] = res.results[c]["y"]
    return out


# revision 11
# speedup vs baseline: 1.1556x; 1.1556x over previous
"""GQA causal attention block (B=2, L=2048, d_model=2048, 32 Q heads / 8 KV heads)
on 8 TRN2 NeuronCores.

Round-3 kernel: software-pipelined attention inner loop (score matmuls of step
k+1 issue before AV of step k so the PE never waits on the scalar-engine exp),
causal column restriction on the AV matmuls (no dead-zone memsets), exp merged
into 1-2 wide ACTIVATE instrs per step, phase-4 normalization prep overlapped
with attention / the second AllToAll, and finer first-chunk x DMA.

Baseline (round 2) measured 468 us on HW; see kernel_r2_backup.py.
"""

import os
import sys
import math

os.environ.setdefault("MYCRO_LOCAL_CACHE", "1")
for _p in ("/opt/trn_rl_repo",):
    if os.path.isdir(_p) and _p not in sys.path:
        sys.path.insert(0, _p)

import numpy as np
import ml_dtypes

import concourse.bass as bass
import concourse.bacc as bacc
import concourse.mybir as mybir
import concourse.tile as tile
from concourse.bass_utils import run_bass_kernel_spmd
from concourse.masks import make_identity

F32 = mybir.dt.float32
BF16 = mybir.dt.bfloat16
Exp = mybir.ActivationFunctionType.Exp

D = 2048
L = 2048
DH = 64
B = 2
NCORES = 8
NH_L = 4
QF = NH_L * DH
LC1 = 512
NLC1 = L // LC1
LT = 512
NT = L // LT
NB = L // 128
SH = 2 * DH  # 128 rows per A2A shard (normalized, no denom rows)
SCALE = 1.0 / math.sqrt(DH)

_CACHE = {}


def _build_nc():
    nc = bacc.Bacc(
        "TRN2",
        target_bir_lowering=False,
        debug=False,
        enable_asserts=False,
        num_devices=NCORES,
    )
    xh0 = nc.dram_tensor("xh0", [NLC1 * 128, 16 * LC1], BF16, kind="ExternalInput")
    xh1 = nc.dram_tensor("xh1", [NLC1 * 128, 16 * LC1], BF16, kind="ExternalInput")
    wqh = nc.dram_tensor("wqh", [128, 16 * QF], BF16, kind="ExternalInput")
    wkvh = nc.dram_tensor("wkvh", [128, 16 * 256], BF16, kind="ExternalInput")
    woh = nc.dram_tensor("woh", [D, D], BF16, kind="ExternalInput")
    y = nc.dram_tensor("y", [LT, D], F32, kind="ExternalOutput")

    with tile.TileContext(nc) as tc:
        with tc.tile_pool(name="dram", bufs=1, space="DRAM") as dram:
            bins = [
                dram.tile([NCORES * SH, LT], BF16, name=f"bin{hp}") for hp in range(2)
            ]
            bouts = [
                dram.tile([NCORES * SH, LT], BF16, name=f"bout{hp}") for hp in range(2)
            ]
            rnorms = [
                (
                    dram.tile([8, 1024], BF16, name=f"rnorm{hp}"),
                    dram.tile([8, 1024], F32, name=f"dnraw{hp}"),
                )
                for hp in range(2)
            ]

            with tc.tile_pool(name="const", bufs=1) as const:
                ident = const.tile([128, 128], BF16, name="ident")
                make_identity(nc, ident)

                with tc.tile_pool(name="pers", bufs=1) as pers:
                    qp = [
                        pers.tile([128, NT * 1024], BF16, name=f"qp{p}")
                        for p in range(2)
                    ]
                    kbT = pers.tile([128, L], BF16, name="kbT")
                    vaug = pers.tile([128, NB * 130], BF16, name="vaug")
                    va = vaug.rearrange("p (b c) -> p b c", c=130)
                    nc.gpsimd.memset(va[:, :, 64:65], 1.0)
                    nc.gpsimd.memset(va[:, :, 129:130], 1.0)

                    _phase1_qkv(nc, tc, xh0, xh1, wqh, wkvh, qp, kbT, va, ident)
                    with tc.tile_pool(name="wo", bufs=1) as wop:
                        wo_sb = wop.tile([128, 16 * D], BF16, name="wo_sb")
                        nc.gpsimd.dma_start(
                            wo_sb.rearrange("p (k d) -> p k d", d=D),
                            woh.rearrange("(k p) d -> p k d", p=128),
                        )
                        with tc.tile_pool(name="an", bufs=1) as anp:
                            ans = {}
                            with (
                                tc.tile_pool(name="p2s", bufs=1, space="PSUM") as scp,
                                tc.tile_pool(name="p2o", bufs=1, space="PSUM") as ovp,
                                tc.tile_pool(name="pbuf", bufs=1) as pbp,
                                tc.tile_pool(name="stg", bufs=3) as stp,
                            ):
                                pools = (scp, ovp, pbp, stp)
                                _attn_pair(nc, tc, pools, qp, kbT, va, bins[0], rnorms[0], 0)
                                nc.gpsimd.collective_compute(
                                    "AllToAll",
                                    mybir.AluOpType.bypass,
                                    ins=[bins[0].opt()],
                                    outs=[bouts[0].opt()],
                                    replica_groups=[list(range(NCORES))],
                                )
                                _attn_pair(
                                    nc, tc, pools, qp, kbT, va, bins[1], rnorms[1], 1,
                                    after_tau={
                                        1: lambda: _prep_half(
                                            nc, anp, bouts, 0, ans
                                        )
                                    },
                                )
                                nc.gpsimd.collective_compute(
                                    "AllToAll",
                                    mybir.AluOpType.bypass,
                                    ins=[bins[1].opt()],
                                    outs=[bouts[1].opt()],
                                    replica_groups=[list(range(NCORES))],
                                )
                            # attention PSUM pools released here
                            _prep_half(nc, anp, bouts, 1, ans)
                            _phase4_oproj(nc, tc, wo_sb, ans, y)
    nc.finalize()
    return nc


def _phase1_qkv(nc, tc, xh0, xh1, wqh, wkvh, qp, kbT, va, ident):
    with (
        tc.tile_pool(name="w1", bufs=1) as wpool,
        tc.tile_pool(name="xc", bufs=2) as xpool,
        tc.tile_pool(name="vt", bufs=2) as vtpool,
        tc.tile_pool(name="p1q", bufs=1, space="PSUM") as p1q,
        tc.tile_pool(name="p1kv", bufs=1, space="PSUM") as p1kv,
    ):
        wq_sb = wpool.tile([128, 16 * QF], BF16, name="wq_sb")
        wkv_sb = wpool.tile([128, 16 * 256], BF16, name="wkv_sb")
        nc.sync.dma_start(wq_sb[:, :], wqh[:, :])
        nc.sync.dma_start(wkv_sb[:, :], wkvh[:, :])

        for lc in range(NLC1):
            x0 = xpool.tile([128, 16 * LC1], BF16, name="x0", tag="x0")
            x1 = xpool.tile([128, 16 * LC1], BF16, name="x1", tag="x1")
            # finer-grained parts for the first chunk so the first matmul
            # can start as soon as one fb-column block has landed
            nparts = 16 if lc == 0 else 4
            pstep = (16 * LC1) // nparts
            for part in range(nparts):
                cs = slice(part * pstep, (part + 1) * pstep)
                nc.gpsimd.dma_start(x0[:, cs], xh0[lc * 128 : (lc + 1) * 128, cs])
                nc.gpsimd.dma_start(x1[:, cs], xh1[lc * 128 : (lc + 1) * 128, cs])
            cols = slice(lc * LC1, (lc + 1) * LC1)

            aq = [
                [
                    p1q.tile([128, LC1], F32, name=f"aq{p}{b}", tag=f"aq{p}{b}")
                    for b in range(2)
                ]
                for p in range(2)
            ]
            akv = [
                p1kv.tile([128, LC1], F32, name=f"akv{b}", tag=f"akv{b}")
                for b in range(2)
            ]
            for fb in range(16):
                st = dict(start=(fb == 0), stop=(fb == 15))
                x0f = x0[:, fb * LC1 : (fb + 1) * LC1]
                x1f = x1[:, fb * LC1 : (fb + 1) * LC1]
                for p in range(2):
                    w = wq_sb[:, fb * QF + p * 128 : fb * QF + (p + 1) * 128]
                    nc.tensor.matmul(aq[p][0][:, :], w, x0f, **st)
                    nc.tensor.matmul(aq[p][1][:, :], w, x1f, **st)
                wkv0 = wkv_sb[:, fb * 256 : fb * 256 + 128]
                wkv1 = wkv_sb[:, fb * 256 + 128 : (fb + 1) * 256]
                nc.tensor.matmul(akv[0][:, :], wkv0, x0f, **st)
                nc.tensor.matmul(akv[1][:, :], wkv1, x1f, **st)

            for p in range(2):
                qc = lc * 1024
                nc.scalar.copy(qp[p][0:64, qc : qc + 512], aq[p][0][0:64, :])
                nc.scalar.copy(qp[p][64:128, qc : qc + 512], aq[p][1][0:64, :])
                nc.scalar.copy(
                    qp[p][64:128, qc + 512 : qc + 1024], aq[p][1][64:128, :]
                )
                tq = vtpool.tile([128, LC1], BF16, name="tq", tag=f"tq{p}")
                nc.scalar.copy(tq[64:128, :], aq[p][0][64:128, :])
                nc.sync.dma_start(qp[p][0:64, qc + 512 : qc + 1024], tq[64:128, :])
            nc.scalar.copy(kbT[0:64, cols], akv[0][0:64, :])
            nc.scalar.copy(kbT[64:128, cols], akv[1][64:128, :])
            vsb = vtpool.tile([128, LC1], BF16, name="vsb", tag="vsb")
            nc.scalar.copy(vsb[0:64, :], akv[1][0:64, :])
            nc.scalar.copy(vsb[64:128, :], akv[0][64:128, :])
            for s in range(LC1 // 128):
                beta = (lc * LC1) // 128 + s
                tp = p1kv.tile([128, 128], BF16, name="tp", tag="tp", bufs=2)
                nc.tensor.matmul(
                    tp[:, :],
                    vsb[:, s * 128 : (s + 1) * 128],
                    ident[:, :],
                    is_transpose=True,
                )
                nc.scalar.copy(va[:, beta, 0:64], tp[:, 64:128])
                nc.scalar.copy(va[:, beta, 65:129], tp[:, 0:64])


def _attn_pair(nc, tc, pools, qp, kbT, va, bin_, rnorms_hp, hp, after_tau=None):
    rnorm, dnraw = rnorms_hp
    """Attention for head pair hp (2 heads x 2 batches), software-pipelined:
    the score matmuls for step k+1 are issued before the AV matmuls of step
    k, so the tensor engine streams while the scalar engine computes exp."""
    scp, ovp, pbp, stp = pools

    steps = []
    for tau in range(NT):
        nb = 4 * tau + 4
        for blk in range(nb):
            for h in (0, 1):
                steps.append((tau, blk, h))

    outs = {}

    def get_out(tau):
        if tau not in outs:
            outs[tau] = (
                ovp.tile([65, 1024], F32, name="oab", tag="oab"),
                ovp.tile([65, 1024], F32, name="obb", tag="obb"),
            )
        return outs[tau]

    def emit_S(step):
        tau, blk, h = step
        off = max(blk - 4 * tau, 0) * 128
        kc = slice(blk * 128, (blk + 1) * 128)
        tq = tau * 1024
        sb = scp.tile([128, 1024], F32, name="sb", tag="sb", bufs=2)
        nc.tensor.matmul(
            sb[:, off:512],
            kbT[64 * h : 64 * (h + 1), kc],
            qp[hp][64 * h : 64 * (h + 1), tq + off : tq + 512],
        )
        nc.tensor.matmul(
            sb[:, 512 + off : 1024],
            kbT[64 * h : 64 * (h + 1), kc],
            qp[hp][64 * h : 64 * (h + 1), tq + 512 + off : tq + 1024],
            skip_group_check=True,
        )
        return sb

    def emit_exp(step, sb):
        tau, blk, h = step
        dj = blk - 4 * tau
        off = max(dj, 0) * 128
        Pb = pbp.tile([128, 1024], BF16, name="Pb", tag="Pb", bufs=3)
        if off == 0:
            nc.scalar.activation(Pb[:, :], sb[:, :], Exp)
        else:
            nc.scalar.activation(Pb[:, off:512], sb[:, off:512], Exp)
            nc.scalar.activation(
                Pb[:, 512 + off : 1024], sb[:, 512 + off : 1024], Exp
            )
        if dj >= 0:
            for base in (0, 512):
                dg = Pb[:, base + off : base + off + 128]
                nc.gpsimd.affine_select(
                    out=dg,
                    in_=dg,
                    compare_op=mybir.AluOpType.is_ge,
                    fill=0.0,
                    base=0,
                    pattern=[[1, 128]],
                    channel_multiplier=-1,
                )
        return Pb

    def emit_AV(step, Pb):
        tau, blk, h = step
        nb = 4 * tau + 4
        off = max(blk - 4 * tau, 0) * 128
        out = get_out(tau)[h]
        st = dict(start=(blk == 0), stop=(blk == nb - 1))
        nc.tensor.matmul(
            out[:, off:512],
            va[:, blk, 65 * h : 65 * h + 65],
            Pb[:, off:512],
            skip_group_check=True,
            **st,
        )
        nc.tensor.matmul(
            out[:, 512 + off : 1024],
            va[:, blk, 65 * h : 65 * h + 65],
            Pb[:, 512 + off : 1024],
            skip_group_check=True,
            **st,
        )

    def emit_evac(tau):
        oab, obb = outs.pop(tau)
        for bb, psrc in ((0, oab), (1, obb)):
            # snapshot the numerators to SBUF and fold the denominator row
            # into 128 partitions, so the PSUM tile rotation is only gated by
            # one fast copy + one tiny DMA; the reciprocal then runs on an
            # 8-column AP (vector time scales with free size, not partitions)
            snapb = stp.tile([64, 1024], BF16, name="snapb", tag=f"snapb{bb}", bufs=2)
            nc.vector.tensor_copy(snapb[:, :], psrc[0:64, :])
            snapd = stp.tile([1, 1024], F32, name="snapd", tag=f"snapd{bb}", bufs=2)
            nc.vector.tensor_copy(snapd[:, :], psrc[64:65, :])
            rr = 2 * tau + bb
            nc.sync.dma_start(dnraw[rr : rr + 1, :], snapd[0:1, :])
            fold = stp.tile([128, 8], F32, name="fold", tag=f"fold{bb}", bufs=2)
            nc.sync.dma_start(
                fold[:, :],
                dnraw[rr : rr + 1, :].rearrange("o (p j) -> (o p) j", j=8),
            )
            rfold = stp.tile([128, 8], BF16, name="rfold", tag=f"rfold{bb}", bufs=2)
            rtmp = stp.tile([128, 8], F32, name="rtmp", tag=f"rtmp{bb}", bufs=2)
            nc.vector.reciprocal(rtmp[:, :], fold[:, :])
            nc.vector.tensor_copy(rfold[:, :], rtmp[:, :])
            nc.sync.dma_start(
                rnorm[rr : rr + 1, :].rearrange("o (p j) -> (o p) j", j=8),
                rfold[:, :],
            )
            rbc = stp.tile([64, 1024], BF16, name="rbc", tag=f"rbc{bb}", bufs=2)
            nc.sync.dma_start(
                rbc[:, :], rnorm[rr : rr + 1, :].partition_broadcast(64)
            )
            stgn = stp.tile([64, 1024], BF16, name="stgn", tag=f"stgn{bb}", bufs=2)
            nc.vector.tensor_mul(stgn[:, :], snapb[:, :], rbc[:, :])
            base = SH * (4 * bb + tau)
            for hh in (0, 1):
                nc.sync.dma_start(
                    bin_[base + 64 * hh : base + 64 * (hh + 1), :],
                    stgn[:, 512 * hh : 512 * (hh + 1)],
                )

    sb_cur = emit_S(steps[0])
    for i, step in enumerate(steps):
        Pb = emit_exp(step, sb_cur)
        if i + 1 < len(steps):
            sb_cur = emit_S(steps[i + 1])
        emit_AV(step, Pb)
        tau, blk, h = step
        nb = 4 * tau + 4
        if blk == nb - 1 and h == 1:
            emit_evac(tau)
            if after_tau and tau in after_tau:
                after_tau[tau]()


def _prep_half(nc, anp, bouts, half, ans):
    """Load one A2A result into SBUF: the numerators arrive already
    normalized, so this is just 8 plain DMA loads on the sync queue."""
    bo = bouts[half]
    for c in range(NCORES):
        k = 2 * c + half
        au = anp.tile([128, LT], BF16, name=f"au{k}", tag=f"au{k}")
        nc.sync.dma_start(au[:, :], bo[SH * c : SH * (c + 1), :])
        ans[k] = au


def _phase4_oproj(nc, tc, wo_sb, ans, y):
    with (
        tc.tile_pool(name="ysum", bufs=1) as ysp,
        tc.tile_pool(name="ysb", bufs=2) as yp,
        tc.tile_pool(name="p4y", bufs=2, space="PSUM") as eyp,
    ):
        ysum = [
            ysp.tile([128, D], F32, name=f"ysum{m}", tag=f"ysum{m}")
            for m in range(4)
        ]
        for half in range(2):
            for m in range(4):
                yps = eyp.tile([128, D], F32, name="yps", tag="yps")
                for ki in range(NCORES):
                    k = 2 * ki + half
                    st = dict(start=(ki == 0), stop=(ki == NCORES - 1))
                    for q in range(4):
                        nc.tensor.matmul(
                            yps[:, q * 512 : (q + 1) * 512],
                            ans[k][:, m * 128 : (m + 1) * 128],
                            wo_sb[:, k * D + q * 512 : k * D + (q + 1) * 512],
                            skip_group_check=(q > 0),
                            **st,
                        )
                if half == 0:
                    nc.scalar.copy(ysum[m][:, :], yps[:, :])
                else:
                    ysb = yp.tile([128, D], F32, name="ysb", tag="ysb")
                    nc.vector.tensor_add(ysb[:, :], yps[:, :], ysum[m][:, :])
                    nc.sync.dma_start(y[m * 128 : (m + 1) * 128, :], ysb[:, :])


def _get_nc():
    if "nc" not in _CACHE:
        _CACHE["nc"] = _build_nc()
    return _CACHE["nc"]


LAST_EXEC_NS = None


def _prep_x(xb):
    xT = xb.T.astype(ml_dtypes.bfloat16)
    v = xT.reshape(16, 128, NLC1, LC1)
    v = v.transpose(2, 1, 0, 3)
    return np.ascontiguousarray(v.reshape(NLC1 * 128, 16 * LC1))


def kernel(x, Wq, Wk, Wv, Wo):
    global LAST_EXEC_NS
    x = np.asarray(x, dtype=np.float32)
    Wq = np.asarray(Wq, dtype=np.float32)
    Wk = np.asarray(Wk, dtype=np.float32)
    Wv = np.asarray(Wv, dtype=np.float32)
    Wo = np.asarray(Wo, dtype=np.float32)

    xh0 = _prep_x(x[0])
    xh1 = _prep_x(x[1])
    woh = np.ascontiguousarray(Wo.T.astype(ml_dtypes.bfloat16))

    in_maps = []
    for c in range(NCORES):
        wq_c = (SCALE * Wq[QF * c : QF * (c + 1), :]).astype(ml_dtypes.bfloat16)
        wqh = np.ascontiguousarray(
            wq_c.reshape(QF, 16, 128).transpose(2, 1, 0).reshape(128, 16 * QF)
        )
        wk_c = Wk[DH * c : DH * (c + 1), :].astype(ml_dtypes.bfloat16)
        wkh = wk_c.reshape(DH, 16, 128).transpose(2, 1, 0)
        wv_c = Wv[DH * c : DH * (c + 1), :].astype(ml_dtypes.bfloat16)
        wvh = wv_c.reshape(DH, 16, 128).transpose(2, 1, 0)
        wkvh = np.empty((128, 16, 256), dtype=ml_dtypes.bfloat16)
        wkvh[:, :, 0:64] = wkh
        wkvh[:, :, 64:128] = wvh
        wkvh[:, :, 128:192] = wvh
        wkvh[:, :, 192:256] = wkh
        wkvh = np.ascontiguousarray(wkvh.reshape(128, 16 * 256))
        in_maps.append(
            {"xh0": xh0, "xh1": xh1, "wqh": wqh, "wkvh": wkvh, "woh": woh}
        )

    nc = _get_nc()
    res = run_bass_kernel_spmd(nc, in_maps, core_ids=list(range(NCORES)))
    LAST_EXEC_NS = getattr(res, "exec_time_ns", None)

    out = np.empty((B, L, D), dtype=np.float32)
    for c in range(NCORES):
        b, g = divmod(c, 4)
        out[b, 512 * g : 512 * (g + 1), :] = res.results[c]["y"]
    return out
